# revision 12
# baseline (speedup 1.0000x reference)
"""DeepGraphInfomax loss (2-layer GCN encoder, pos+neg, DGI readout) on 8 trn2 cores.

Window-major dst-sharded pull-mode GNN aggregation:
  - Nodes (dst rows) sharded contiguously across 8 cores (12500 each).
  - pos/neg feature streams fused into 128-wide rows: X2[r] = [x[r] | x[perm[r]]].
  - Self-loops folded in as explicit edges with degree product deg^2, so the
    aggregation produces the complete GCN pre-activation in one pass.
  - Source rows live in a quarter-major layout: node (core k, local l) maps to
    row 25600*(l//3200) + 3200*k + (l%3200).  The 4 sections of 25600 rows keep
    int16 gather indices valid, AND layer-1 (x2q) and layer-2 (r2full) share
    the exact same index space, so idx/dstl/norm arrays are staged and loaded
    once for both layers.
  - Processing is window-major: all tiles of one 128-dst window (across all 4
    source sections) accumulate into a single PSUM tile via one-hot matmuls
    with swapped operands (lhsT=gathered rows, rhs=one-hot), yielding
    feature-major results directly.  No DRAM accumulator, no scatter-add.
  - post per window: ACT copies psum->SBUF, PE applies W (A @ (X W) == (A @ X) W),
    ACT applies bias(+relu).  Layer-1 results are DMA-transposed to row-major
    bf16 and stored to r2shard.
  - r2shard is AllGathered in 4 quarter chunks, each gated only on the quarter
    of post-L1 windows it needs, so layer-2 gathers start while layer-1 post
    is still finishing.
  - DGI readout (summary / W_dgi / softplus losses) computed on device with two
    tiny AllReduces.

Host-side preprocessing only manipulates integer graph structure (sorting,
degree counts, packing, index mapping) and stages dtype-cast copies of the
inputs; all floating-point math of the reference runs on device.
"""

import sys

for _p in ("/opt/trn_rl_repo", "/root/.axon_site/_ro/trn_rl_repo"):
    if _p not in sys.path:
        sys.path.insert(0, _p)

from contextlib import ExitStack

import ml_dtypes
import numpy as np

import concourse.bass as bass
import concourse.bacc as bacc
import concourse.mybir as mybir
import concourse.tile as tile
from concourse.bass_utils import run_bass_kernel_spmd

BF16 = ml_dtypes.bfloat16
F32 = np.float32

C = 8            # cores
D = 64           # hidden dim
DF = 2 * D       # fused pos|neg width
NSEC = 4
TILES_PER_CALL = 32
SLOTS_PER_CALL = TILES_PER_CALL * 128
PAD_DEG = 1e30   # pad-slot degree product -> norm ~ 1e-15 ~ 0


class Geo:
    def __init__(self, npc, nreal):
        self.npc = npc                       # real nodes per core
        self.nreal = nreal                   # total real nodes (= 8*npc)
        self.nw = -(-npc // 128)             # dst windows per core (98)
        self.ldim = 128 * self.nw            # padded dsts per core (12544)
        self.wpq = -(-self.nw // NSEC)       # windows per quarter (25)
        self.ql = self.wpq * 128             # locals per quarter (3200)
        self.sec = C * self.ql               # rows per section (25600)
        self.xrows = NSEC * self.sec         # padded source-row space (102400)
        self.shard = NSEC * self.ql          # r2shard rows (12800)
        assert self.sec < 32768


def _preprocess(g, x, W1, b1, W2, b2, W_dgi, edge_index, perm):
    """Build per-core device inputs. Integer index work + dtype staging only."""
    row = np.asarray(edge_index[0], dtype=np.int64)
    col = np.asarray(edge_index[1], dtype=np.int64)
    perm = np.asarray(perm, dtype=np.int64)
    N = g.nreal
    npc, ql = g.npc, g.ql

    deg = np.bincount(col, minlength=N).astype(np.int64) + 1  # in-deg + 1

    # quarter-major source-row id per global node
    gids = np.arange(N, dtype=np.int64)
    kk = gids // npc
    ll = gids % npc
    r2p = g.sec * (ll // ql) + ql * kk + (ll % ql)

    # fused bf16 feature rows in quarter-major layout
    X2 = np.zeros((g.xrows, DF), dtype=BF16)
    X2[r2p, :D] = x.astype(BF16)
    X2[r2p, D:] = x[perm].astype(BF16)

    # edges + self-loops (self: src == dst, degp = deg^2 -> weight 1/deg)
    rows_a = np.concatenate([row, gids])
    cols_a = np.concatenate([col, gids])
    src_q = r2p[rows_a]                       # quarter-major src row
    kd = cols_a // npc                        # dst core
    dl = cols_a % npc                         # dst local
    sec = src_q // g.sec
    w = dl // 128

    # tile counts per (core, sec, window) -> T = max over cores
    key = ((kd * NSEC + sec) * g.nw + w).astype(np.int64)
    cnt = np.bincount(key, minlength=C * NSEC * g.nw).reshape(C, NSEC, g.nw)
    T = np.maximum(-(-cnt // 128), 0).max(axis=0)           # [NSEC, NW]
    tiles_s = T.sum(axis=1)                                 # tiles per section
    ntiles = int(tiles_s.sum())
    calls = [
        [
            TILES_PER_CALL
            if (c + 1) * TILES_PER_CALL <= tiles_s[s]
            else int(tiles_s[s] - c * TILES_PER_CALL)
            for c in range(-(-int(tiles_s[s]) // TILES_PER_CALL))
        ]
        for s in range(NSEC)
    ]
    tbase = np.concatenate([[0], np.cumsum(tiles_s)])       # section tile base
    # slot base of each (s, w) run
    wbase = np.zeros((NSEC, g.nw), dtype=np.int64)
    for s in range(NSEC):
        wbase[s] = (tbase[s] + np.concatenate([[0], np.cumsum(T[s])[:-1]])) * 128

    deg_f = deg.astype(np.float64)
    degp_a = deg_f[rows_a] * deg_f[cols_a]

    ins = []
    for k in range(C):
        m = kd == k
        sq, dk, wk, sk = src_q[m], dl[m], w[m], sec[m]
        dp = degp_a[m]
        order = np.lexsort((sq, dk, wk, sk))
        sq, dk, wk, sk, dp = (a[order] for a in (sq, dk, wk, sk, dp))
        # rank within each (sec, window) run
        runkey = sk * g.nw + wk
        starts = np.searchsorted(runkey, runkey, side="left")
        rank = np.arange(len(runkey)) - starts
        slot = wbase[sk, wk] + rank

        S = ntiles * 128
        idx = np.zeros(S, dtype=np.int16)
        dstl = np.zeros(S, dtype=np.int32)
        degp = np.full(S, PAD_DEG, dtype=F32)
        idx[slot] = (sq - sk * g.sec).astype(np.int16)
        dstl[slot] = dk - wk * 128
        degp[slot] = dp.astype(F32)
        assert dstl.min() >= 0 and dstl.max() < 128

        d_in = {
            # wrapped int16 index layout: slot j -> [j%16, j//16], replicated x8
            "idx": np.ascontiguousarray(
                np.tile(idx.reshape(-1, 16).T, (8, 1)).astype(np.int16)
            ),
            "dstl": np.ascontiguousarray(dstl.reshape(-1, 128).T.astype(F32)),
            "degp": np.ascontiguousarray(degp.reshape(-1, 128).T),
        }
        ins.append(d_in)

    # shared constants
    iota = np.tile(np.arange(128, dtype=F32), (128, 1)).astype(BF16)
    wc1 = np.zeros((DF, DF), dtype=F32)
    wc1[:D, :D] = W1
    wc1[D:, D:] = W1
    wc2 = np.zeros((DF, DF), dtype=F32)
    wc2[:D, :D] = W2
    wc2[D:, D:] = W2
    bc1 = np.concatenate([b1, b1]).astype(F32).reshape(DF, 1)
    bc2 = np.concatenate([b2, b2]).astype(F32).reshape(DF, 1)
    wstack = np.zeros((D, DF), dtype=F32)
    wstack[:, :D] = W_dgi.T
    wstack[:, D:] = W_dgi.T
    colmask = np.zeros((DF, 2), dtype=F32)
    colmask[:D, 0] = 1.0
    colmask[D:, 1] = 1.0
    nvalid_last = g.npc - (g.nw - 1) * 128
    lastmask = np.tile((np.arange(128) < nvalid_last).astype(F32), (128, 1))
    mk = (np.arange(g.ldim) < g.npc).astype(F32)
    shared = {
        "x2": X2,
        "iota": iota,
        "wc1": wc1,
        "wc2": wc2,
        "bc1": bc1,
        "bc2": bc2,
        "wstack": wstack,
        "colmask": colmask,
        "lastmask": lastmask,
        "mask": np.ascontiguousarray(mk.reshape(g.nw, 128).T),
        "ones": np.ones((128, 1), dtype=F32),
    }
    for d_in in ins:
        d_in.update(shared)
    struct = (tuple(map(tuple, T)), tuple(map(tuple, calls)))
    return ins, struct


def _build(g, struct):
    T, calls = struct
    T = [list(r) for r in T]
    calls = [list(r) for r in calls]
    tiles_s = [sum(r) for r in T]
    ntiles = sum(tiles_s)
    tbase = [0]
    for s in range(NSEC):
        tbase.append(tbase[-1] + tiles_s[s])

    dt = mybir.dt
    nc = bacc.Bacc(
        "TRN2", target_bir_lowering=False, debug=False, num_devices=C
    )

    def din(name, shape, dty):
        return nc.dram_tensor(name, list(shape), dty, kind="ExternalInput").ap()

    x2 = din("x2", (g.xrows, DF), dt.bfloat16)
    idx_d = din("idx", (128, ntiles * 8), dt.int16)
    dstl_d = din("dstl", (128, ntiles), dt.float32)
    degp_d = din("degp", (128, ntiles), dt.float32)
    iota_d = din("iota", (128, 128), dt.bfloat16)
    wc_d = [din("wc1", (DF, DF), dt.float32), din("wc2", (DF, DF), dt.float32)]
    bc_d = [din("bc1", (DF, 1), dt.float32), din("bc2", (DF, 1), dt.float32)]
    wstack_d = din("wstack", (D, DF), dt.float32)
    colmask_d = din("colmask", (DF, 2), dt.float32)
    lastmask_d = din("lastmask", (128, 128), dt.float32)
    mask_d = din("mask", (128, g.nw), dt.float32)
    ones_d = din("ones", (128, 1), dt.float32)
    loss_out = nc.dram_tensor("loss", [1, 16], dt.float32, kind="ExternalOutput").ap()

    inv_n = 1.0 / float(g.nreal)
    rg = [list(range(C))]

    with tile.TileContext(nc) as tc, ExitStack() as ctx:
        dram = ctx.enter_context(tc.tile_pool(name="dram", bufs=1, space="DRAM"))
        r2shard = []
        for j in range(NSEC):
            r2s_j = dram.tile(
                [g.ql, DF], dt.bfloat16, tag=f"r2shard{j}", name=f"r2shard_{j}"
            )
            r2shard.append(r2s_j)
        r2full = []
        for j in range(NSEC):
            r2f_j = dram.tile(
                [g.sec, DF], dt.bfloat16, tag=f"r2full{j}",
                addr_space="Shared", name=f"r2full_{j}",
            )
            r2full.append(r2f_j)
        cs_in = dram.tile([128, 1], dt.float32, tag="cs_in")
        cs_out = dram.tile([128, 1], dt.float32, tag="cs_out", addr_space="Shared")
        ls_in = dram.tile([1, 16], dt.float32, tag="ls_in")
        ls_out = dram.tile([1, 16], dt.float32, tag="ls_out", addr_space="Shared")

        const = ctx.enter_context(tc.tile_pool(name="const", bufs=1))

        def cload(ap_dram, shape, dty, tag):
            t = const.tile(list(shape), dty, tag=tag)
            nc.sync.dma_start(t[:], ap_dram)
            return t

        iota_sb = cload(iota_d, (128, 128), dt.bfloat16, "iota")
        wc_sb = [
            cload(wc_d[0], (DF, DF), dt.float32, "wc1"),
            cload(wc_d[1], (DF, DF), dt.float32, "wc2"),
        ]
        bc_sb = [
            cload(bc_d[0], (DF, 1), dt.float32, "bc1"),
            cload(bc_d[1], (DF, 1), dt.float32, "bc2"),
        ]
        wstack_sb = cload(wstack_d, (D, DF), dt.float32, "wstack")
        colmask_sb = cload(colmask_d, (DF, 2), dt.float32, "colmask")
        lastmask_sb = cload(lastmask_d, (128, 128), dt.float32, "lastmask")
        mask_sb = cload(mask_d, (128, g.nw), dt.float32, "mask")
        ones_sb = cload(ones_d, (128, 1), dt.float32, "ones")

        big = ctx.enter_context(tc.tile_pool(name="big", bufs=1))
        z_sb = big.tile([128, g.ldim], dt.float32, tag="z_sb")
        idx_all = big.tile([128, ntiles * 8], dt.int16, tag="idx_all")
        nc.sync.dma_start(idx_all[:], idx_d)
        dl_sb = big.tile([128, ntiles], dt.float32, tag="dl_sb")
        nc.sync.dma_start(dl_sb[:], dstl_d)
        wv_sb = big.tile([128, ntiles], dt.float32, tag="wv_sb")
        nc.sync.dma_start(wv_sb[:], degp_d)
        nc.vector.reciprocal(wv_sb[:], wv_sb[:])
        nc.scalar.sqrt(wv_sb[:], wv_sb[:])

        gpool = ctx.enter_context(tc.tile_pool(name="gpool", bufs=6))
        ppool = ctx.enter_context(tc.tile_pool(name="ppool", bufs=6))
        psw = ctx.enter_context(tc.tile_pool(name="psw", bufs=3, space="PSUM"))
        psm = ctx.enter_context(tc.tile_pool(name="psm", bufs=2, space="PSUM"))
        psl = ctx.enter_context(tc.tile_pool(name="psl", bufs=1, space="PSUM"))
        upool = ctx.enter_context(tc.tile_pool(name="upool", bufs=3))
        outp = ctx.enter_context(tc.tile_pool(name="outp", bufs=4))

        # per-section call start tiles
        call_start = [
            [c * TILES_PER_CALL for c in range(len(calls[s]))] for s in range(NSEC)
        ]

        def layer(li, src_of, emit_ag):
            gts = [{} for _ in range(NSEC)]     # sec -> call -> (tile, ntile)

            def ensure_call(s, ci):
                if ci in gts[s] or ci >= len(calls[s]):
                    return
                nt = calls[s][ci]
                gt = gpool.tile([128, TILES_PER_CALL, DF], dt.bfloat16, tag="gt")
                gtile = tbase[s] + call_start[s][ci]
                src_sec = src_of(s)
                nc.gpsimd.dma_gather(
                    gt[:, :nt, :],
                    src_sec,
                    idx_all[:, gtile * 8 : (gtile + nt) * 8],
                    nt * 128,
                    nt * 128,
                    DF,
                    single_packet=False,
                )
                gts[s][ci] = (gt, nt)

            for s in range(NSEC):
                ensure_call(s, 0)

            cursor = [0] * NSEC                 # consumed tiles per sec
            for w in range(g.nw):
                nt_w = sum(T[s][w] for s in range(NSEC))
                # prefetch: make sure calls covering this window (+1 ahead) exist
                for s in range(NSEC):
                    if T[s][w] == 0:
                        continue
                    clast = (cursor[s] + T[s][w] - 1) // TILES_PER_CALL
                    for ci in range(clast + 2):
                        ensure_call(s, ci)
                ps = psw.tile([128, 128], dt.float32, tag="ps")
                ti = 0
                for s in range(NSEC):
                    for t in range(T[s][w]):
                        c = cursor[s] + t
                        ci, sl = c // TILES_PER_CALL, c % TILES_PER_CALL
                        gt, _ = gts[s][ci]
                        col = tbase[s] + c
                        P = ppool.tile([128, 128], dt.bfloat16, tag="P")
                        nc.vector.tensor_scalar(
                            P[:],
                            iota_sb[:],
                            dl_sb[:, col : col + 1],
                            wv_sb[:, col : col + 1],
                            mybir.AluOpType.is_equal,
                            mybir.AluOpType.mult,
                        )
                        nc.tensor.matmul(
                            ps[:],
                            lhsT=gt[:, sl, :],
                            rhs=P[:],
                            start=(ti == 0),
                            stop=(ti == nt_w - 1),
                        )
                        ti += 1
                for s in range(NSEC):
                    cursor[s] += T[s][w]
                if nt_w == 0:
                    nc.vector.memset(ps[:], 0.0)

                # post: u = psum; out = wc^T @ u; + bias (+relu)
                u = upool.tile([128, 128], dt.float32, tag="u")
                nc.scalar.activation(
                    u[:], ps[:], mybir.ActivationFunctionType.Copy
                )
                po = psm.tile([128, 128], dt.float32, tag="po")
                nc.tensor.matmul(
                    po[:], lhsT=wc_sb[li][:], rhs=u[:], start=True, stop=True
                )
                sl128 = slice(w * 128, (w + 1) * 128)
                if li == 0:
                    rb = outp.tile([128, 128], dt.bfloat16, tag="rb")
                    nc.scalar.activation(
                        rb[:],
                        po[:],
                        mybir.ActivationFunctionType.Relu,
                        bias=bc_sb[0][:],
                    )
                    rt = outp.tile([128, 128], dt.bfloat16, tag="rt")
                    nc.sync.dma_start_transpose(rt[:], rb[:])
                    q = min(w // g.wpq, NSEC - 1)
                    lw = w - q * g.wpq
                    nc.sync.dma_start(
                        r2shard[q][lw * 128 : (lw + 1) * 128, :], rt[:]
                    )
                else:
                    nc.vector.tensor_scalar(
                        z_sb[:, sl128],
                        po[:],
                        bc_sb[1][:],
                        None,
                        mybir.AluOpType.add,
                    )
                    if w == g.nw - 1:
                        nc.vector.tensor_tensor(
                            z_sb[:, sl128],
                            z_sb[:, sl128],
                            lastmask_sb[:],
                            op=mybir.AluOpType.mult,
                        )
                if emit_ag:
                    for j in range(NSEC):
                        if w + 1 == min((j + 1) * g.wpq, g.nw):
                            nc.gpsimd.collective_compute(
                                "AllGather",
                                mybir.AluOpType.bypass,
                                replica_groups=rg,
                                ins=[r2shard[j][:].opt()],
                                outs=[r2full[j][:].opt()],
                            )

        layer(0, lambda s: x2[s * g.sec : (s + 1) * g.sec, :], emit_ag=True)
        layer(1, lambda s: r2full[s][:], emit_ag=False)

        # ---- DGI readout ----
        fin = ctx.enter_context(tc.tile_pool(name="fin", bufs=1))
        cs = fin.tile([128, 1], dt.float32, tag="cs")
        nc.vector.reduce_sum(cs[:], z_sb[:], axis=mybir.AxisListType.X)
        nc.sync.dma_start(cs_in[:], cs[:])
        nc.gpsimd.collective_compute(
            "AllReduce",
            mybir.AluOpType.add,
            replica_groups=rg,
            ins=[cs_in[:].opt()],
            outs=[cs_out[:].opt()],
        )
        cst = fin.tile([128, 1], dt.float32, tag="cst")
        nc.sync.dma_start(cst[:], cs_out[:])
        summ = fin.tile([128, 1], dt.float32, tag="summ")
        nc.scalar.activation(
            summ[:], cst[:], mybir.ActivationFunctionType.Sigmoid, scale=inv_n
        )
        wsps = psl.tile([DF, 1], dt.float32, tag="pls")
        nc.tensor.matmul(
            wsps[:], lhsT=wstack_sb[:], rhs=summ[0:D, 0:1], start=True, stop=True
        )
        ws2 = fin.tile([DF, 2], dt.float32, tag="ws2")
        nc.vector.tensor_tensor(
            ws2[:],
            colmask_sb[:],
            wsps[:].to_broadcast([DF, 2]),
            op=mybir.AluOpType.mult,
        )
        tp_sb = fin.tile([128, g.nw], dt.float32, tag="tp_sb")
        tn_sb = fin.tile([128, g.nw], dt.float32, tag="tn_sb")
        for dti in range(g.nw):
            sl = slice(dti * 128, (dti + 1) * 128)
            tps = psl.tile([128, 2], dt.float32, tag="pls")
            nc.tensor.matmul(
                tps[:], lhsT=z_sb[:, sl], rhs=ws2[:], start=True, stop=True
            )
            nc.vector.tensor_copy(tp_sb[:, dti : dti + 1], tps[:, 0:1])
            nc.vector.tensor_copy(tn_sb[:, dti : dti + 1], tps[:, 1:2])

        # softplus(sgn*t) = relu(sgn*t) + ln1p(exp(-|t|)); deg-7 poly for ln1p
        LN1P = [
            5.62195900721818e-07, 0.9999574870750696, -0.4992065685478763,
            0.32697310001391783, -0.2228362583278401, 0.13076503250360005,
            -0.05262485136716543, 0.010119082927575069,
        ]

        def softplus_of(t_in, sgn, tagp):
            neg = fin.tile([128, g.nw], dt.float32, tag=f"{tagp}neg")
            nc.vector.tensor_scalar(
                neg[:], t_in[:], -1.0, None, mybir.AluOpType.mult
            )
            ab = fin.tile([128, g.nw], dt.float32, tag=f"{tagp}ab")
            nc.vector.tensor_tensor(ab[:], t_in[:], neg[:], op=mybir.AluOpType.max)
            uu = fin.tile([128, g.nw], dt.float32, tag=f"{tagp}uu")
            nc.scalar.activation(
                uu[:], ab[:], mybir.ActivationFunctionType.Exp, scale=-1.0
            )
            pp_ = fin.tile([128, g.nw], dt.float32, tag=f"{tagp}pp")
            nc.vector.tensor_scalar(
                pp_[:], uu[:], LN1P[7], LN1P[6],
                mybir.AluOpType.mult, mybir.AluOpType.add,
            )
            pm = fin.tile([128, g.nw], dt.float32, tag=f"{tagp}pm")
            for ci in range(5, -1, -1):
                nc.vector.tensor_tensor(
                    pm[:], pp_[:], uu[:], op=mybir.AluOpType.mult
                )
                nc.vector.tensor_scalar(
                    pp_[:], pm[:], LN1P[ci], None, mybir.AluOpType.add
                )
            rl = fin.tile([128, g.nw], dt.float32, tag=f"{tagp}rl")
            nc.vector.tensor_scalar(
                rl[:], (t_in if sgn > 0 else neg)[:], 0.0, None,
                mybir.AluOpType.max,
            )
            res = fin.tile([128, g.nw], dt.float32, tag=f"{tagp}res")
            nc.vector.tensor_tensor(res[:], rl[:], pp_[:], op=mybir.AluOpType.add)
            return res

        spp = softplus_of(tp_sb, -1, "sp")   # softplus(-t_pos)
        spn = softplus_of(tn_sb, +1, "sn")   # softplus(t_neg)
        ssum = fin.tile([128, g.nw], dt.float32, tag="ssum")
        nc.vector.tensor_tensor(ssum[:], spp[:], spn[:], op=mybir.AluOpType.add)
        nc.vector.tensor_tensor(
            ssum[:], ssum[:], mask_sb[:], op=mybir.AluOpType.mult
        )
        srow = fin.tile([128, 1], dt.float32, tag="srow")
        nc.vector.reduce_sum(srow[:], ssum[:], axis=mybir.AxisListType.X)
        tot = psl.tile([1, 1], dt.float32, tag="pls")
        nc.tensor.matmul(
            tot[:], lhsT=srow[:], rhs=ones_sb[:], start=True, stop=True
        )
        lsb = fin.tile([1, 16], dt.float32, tag="lsb")
        nc.vector.memset(lsb[:], 0.0)
        nc.vector.tensor_copy(lsb[0:1, 0:1], tot[:])
        nc.sync.dma_start(ls_in[:], lsb[:])
        nc.gpsimd.collective_compute(
            "AllReduce",
            mybir.AluOpType.add,
            replica_groups=rg,
            ins=[ls_in[:].opt()],
            outs=[ls_out[:].opt()],
        )
        lsf = fin.tile([1, 16], dt.float32, tag="lsf")
        nc.sync.dma_start(lsf[:], ls_out[:])
        lout = fin.tile([1, 16], dt.float32, tag="lout")
        nc.scalar.activation(
            lout[:], lsf[:], mybir.ActivationFunctionType.Copy, scale=inv_n
        )
        nc.sync.dma_start(loss_out, lout[:])

    nc.compile()
    return nc


_prog_cache = {}


def _get_prog(g, struct):
    key = (g.npc, g.nreal, struct)
    if key not in _prog_cache:
        _prog_cache[key] = _build(g, struct)
    return _prog_cache[key]


def run(inputs, npc, nreal, trace=False):
    g = Geo(npc, nreal)
    in_maps, struct = _preprocess(g, **inputs)
    nc = _get_prog(g, struct)
    res = run_bass_kernel_spmd(
        nc, in_maps, core_ids=list(range(C)), trace=trace
    )
    loss = res.results[0]["loss"][0, 0]
    return np.float32(loss), res


def kernel(**inputs):
    out, _ = run(inputs, npc=12500, nreal=100000)
    return out


def _make_sharded_exec(nc, in_maps, reps=1):
    """Reusable jitted shard_map executor mirroring bass2jax's multi-core
    path, with device-resident inputs."""
    import jax
    from jax.experimental.shard_map import shard_map
    from jax.sharding import Mesh, NamedSharding, PartitionSpec

    from concourse import bass2jax, mybir as _mb

    bass2jax.install_neuronx_cc_hook()
    partition_name = (
        nc.partition_id_tensor.name if nc.partition_id_tensor else None
    )
    in_names, out_names, out_avals, zero_shapes = [], [], [], []
    for alloc in nc.m.functions[0].allocations:
        if not isinstance(alloc, _mb.MemoryLocationSet):
            continue
        name = alloc.memorylocations[0].name
        if alloc.kind == "ExternalInput":
            if name != partition_name:
                in_names.append(name)
        elif alloc.kind == "ExternalOutput":
            shape = tuple(alloc.tensor_shape)
            dty = _mb.dt.np(alloc.dtype)
            out_names.append(name)
            out_avals.append(jax.core.ShapedArray(shape, dty))
            zero_shapes.append((shape, dty))
    n_params = len(in_names)
    n_outs = len(out_avals)
    all_names = list(in_names) + list(out_names)
    if partition_name is not None:
        all_names.append(partition_name)
    donate = tuple(range(n_params, n_params + n_outs * reps))

    assert reps == 1  # the neuronx_cc hook allows one bass_exec per module

    def _body(*args):
        operands = list(args)
        if partition_name is not None:
            operands.append(bass2jax.partition_id_tensor())
        outs = bass2jax._bass_exec_p.bind(
            *operands,
            out_avals=tuple(out_avals),
            in_names=tuple(all_names),
            out_names=tuple(out_names),
            lowering_input_output_aliases=(),
            sim_require_finite=True,
            sim_require_nnan=True,
            nc=nc,
        )
        return tuple(outs)

    devices = jax.devices()[:C]
    mesh = Mesh(np.array(devices), ("core",))
    spec = PartitionSpec("core")
    sharded = jax.jit(
        shard_map(
            _body,
            mesh=mesh,
            in_specs=(spec,) * (n_params + n_outs * reps),
            out_specs=(spec,) * n_outs,
            check_rep=False,
        ),
        donate_argnums=donate,
        keep_unused=True,
    )
    shard = NamedSharding(mesh, spec)
    concat_in = [
        jax.device_put(
            np.concatenate([np.asarray(m[nm]) for m in in_maps], axis=0), shard
        )
        for nm in in_names
    ]

    def launch():
        zeros = [
            jax.device_put(np.zeros((C * s[0], *s[1:]), d), shard)
            for (s, d) in zero_shapes
        ]
        return sharded(*concat_in, *zeros)

    def fetch(outs):
        jax.block_until_ready(outs)
        return {
            nm: np.asarray(outs[i]).reshape(C, *out_avals[i].shape)[0]
            for i, nm in enumerate(out_names)
        }

    def run_once():
        return fetch(launch())

    run_once.launch = launch
    run_once.fetch = fetch
    return run_once


def bench(inputs, npc=12500, nreal=100000, iters=6):
    import time

    g = Geo(npc, nreal)
    t0 = time.time()
    in_maps, struct = _preprocess(g, **inputs)
    t1 = time.time()
    nc = _get_prog(g, struct)
    t2 = time.time()
    run_1 = _make_sharded_exec(nc, in_maps)
    out = run_1()  # warmup: compiles + loads NEFF
    t3 = time.time()
    t1s = []
    for _ in range(iters):
        ta = time.time()
        out = run_1()
        t1s.append(time.time() - ta)
    K = 48
    ta = time.time()
    pend = [run_1.launch() for _ in range(K)]
    import jax as _jax
    _jax.block_until_ready(pend)
    tK = time.time() - ta
    per = (tK - min(t1s)) / (K - 1)
    print(
        f"preprocess {t1-t0:.1f}s  build {t2-t1:.1f}s  warmup {t3-t2:.1f}s\n"
        f"  1-shot ms: {[round(t*1e3,2) for t in t1s]}\n"
        f"  {K} pipelined: total {tK*1e3:.1f} ms -> marginal {per*1e3:.3f} ms"
    )
    return np.float32(out["loss"][0, 0]), per


# revision 17
# speedup vs baseline: 1.0152x; 1.0152x over previous
"""DeepGraphInfomax loss (2-layer GCN encoder, pos+neg, DGI readout) on 8 trn2 cores.

Window-major dst-sharded pull-mode GNN aggregation:
  - Nodes (dst rows) sharded contiguously across 8 cores (12500 each).
  - pos/neg feature streams fused into 128-wide rows: X2[r] = [x[r] | x[perm[r]]].
  - Self-loops folded in as explicit edges with degree product deg^2, so the
    aggregation produces the complete GCN pre-activation in one pass.
  - Source rows live in a quarter-major layout: node (core k, local l) maps to
    row 25600*(l//3200) + 3200*k + (l%3200).  The 4 sections of 25600 rows keep
    int16 gather indices valid, AND layer-1 (x2q) and layer-2 (r2full) share
    the exact same index space, so idx/dstl/norm arrays are staged and loaded
    once for both layers.
  - Processing is window-major: all tiles of one 128-dst window (across all 4
    source sections) accumulate into a single PSUM tile via one-hot matmuls
    with swapped operands (lhsT=gathered rows, rhs=one-hot), yielding
    feature-major results directly.  No DRAM accumulator, no scatter-add.
  - post per window: ACT copies psum->SBUF, PE applies W (A @ (X W) == (A @ X) W),
    ACT applies bias(+relu).  Layer-1 results are DMA-transposed to row-major
    bf16 and stored to r2shard.
  - r2shard is AllGathered in 4 quarter chunks, each gated only on the quarter
    of post-L1 windows it needs, so layer-2 gathers start while layer-1 post
    is still finishing.
  - DGI readout (summary / W_dgi / softplus losses) computed on device with two
    tiny AllReduces.

Host-side preprocessing only manipulates integer graph structure (sorting,
degree counts, packing, index mapping) and stages dtype-cast copies of the
inputs; all floating-point math of the reference runs on device.
"""

import sys

for _p in ("/opt/trn_rl_repo", "/root/.axon_site/_ro/trn_rl_repo"):
    if _p not in sys.path:
        sys.path.insert(0, _p)

from contextlib import ExitStack

import ml_dtypes
import numpy as np

import concourse.bass as bass
import concourse.bacc as bacc
import concourse.mybir as mybir
import concourse.tile as tile
from concourse.bass_utils import run_bass_kernel_spmd

BF16 = ml_dtypes.bfloat16
F32 = np.float32

C = 8            # cores
D = 64           # hidden dim
DF = 2 * D       # fused pos|neg width
NSEC = 4
TILES_PER_CALL = 32
SLOTS_PER_CALL = TILES_PER_CALL * 128
PAD_DEG = 1e30   # pad-slot degree product -> norm ~ 1e-15 ~ 0


class Geo:
    def __init__(self, npc, nreal):
        self.npc = npc                       # real nodes per core
        self.nreal = nreal                   # total real nodes (= 8*npc)
        self.nw = -(-npc // 128)             # dst windows per core (98)
        self.ldim = 128 * self.nw            # padded dsts per core (12544)
        self.wpq = -(-self.nw // NSEC)       # windows per quarter (25)
        self.ql = self.wpq * 128             # locals per quarter (3200)
        self.sec = C * self.ql               # rows per section (25600)
        self.xrows = NSEC * self.sec         # padded source-row space (102400)
        self.shard = NSEC * self.ql          # r2shard rows (12800)
        assert self.sec < 32768


def _preprocess(g, x, W1, b1, W2, b2, W_dgi, edge_index, perm):
    """Build per-core device inputs. Integer index work + dtype staging only."""
    row = np.asarray(edge_index[0], dtype=np.int64)
    col = np.asarray(edge_index[1], dtype=np.int64)
    perm = np.asarray(perm, dtype=np.int64)
    N = g.nreal
    npc, ql = g.npc, g.ql

    deg = np.bincount(col, minlength=N).astype(np.int64) + 1  # in-deg + 1

    # quarter-major source-row id per global node
    gids = np.arange(N, dtype=np.int64)
    kk = gids // npc
    ll = gids % npc
    r2p = g.sec * (ll // ql) + ql * kk + (ll % ql)

    # fused bf16 feature rows in quarter-major layout
    X2 = np.zeros((g.xrows, DF), dtype=BF16)
    X2[r2p, :D] = x.astype(BF16)
    X2[r2p, D:] = x[perm].astype(BF16)

    # edges + self-loops (self: src == dst, degp = deg^2 -> weight 1/deg)
    rows_a = np.concatenate([row, gids])
    cols_a = np.concatenate([col, gids])
    src_q = r2p[rows_a]                       # quarter-major src row
    kd = cols_a // npc                        # dst core
    dl = cols_a % npc                         # dst local
    sec = src_q // g.sec
    w = dl // 128

    # tile counts per (core, sec, window) -> T = max over cores
    key = ((kd * NSEC + sec) * g.nw + w).astype(np.int64)
    cnt = np.bincount(key, minlength=C * NSEC * g.nw).reshape(C, NSEC, g.nw)
    T = np.maximum(-(-cnt // 128), 0).max(axis=0)           # [NSEC, NW]
    tiles_s = T.sum(axis=1)                                 # tiles per section
    ntiles = int(tiles_s.sum())
    calls = [
        [
            TILES_PER_CALL
            if (c + 1) * TILES_PER_CALL <= tiles_s[s]
            else int(tiles_s[s] - c * TILES_PER_CALL)
            for c in range(-(-int(tiles_s[s]) // TILES_PER_CALL))
        ]
        for s in range(NSEC)
    ]
    tbase = np.concatenate([[0], np.cumsum(tiles_s)])       # section tile base
    # slot base of each (s, w) run
    wbase = np.zeros((NSEC, g.nw), dtype=np.int64)
    for s in range(NSEC):
        wbase[s] = (tbase[s] + np.concatenate([[0], np.cumsum(T[s])[:-1]])) * 128

    deg_f = deg.astype(np.float64)
    degp_a = deg_f[rows_a] * deg_f[cols_a]

    ins = []
    for k in range(C):
        m = kd == k
        sq, dk, wk, sk = src_q[m], dl[m], w[m], sec[m]
        dp = degp_a[m]
        order = np.lexsort((sq, dk, wk, sk))
        sq, dk, wk, sk, dp = (a[order] for a in (sq, dk, wk, sk, dp))
        # rank within each (sec, window) run
        runkey = sk * g.nw + wk
        starts = np.searchsorted(runkey, runkey, side="left")
        rank = np.arange(len(runkey)) - starts
        slot = wbase[sk, wk] + rank

        S = ntiles * 128
        idx = np.zeros(S, dtype=np.int16)
        dstl = np.zeros(S, dtype=np.int32)
        degp = np.full(S, PAD_DEG, dtype=F32)
        idx[slot] = (sq - sk * g.sec).astype(np.int16)
        dstl[slot] = dk - wk * 128
        degp[slot] = dp.astype(F32)
        assert dstl.min() >= 0 and dstl.max() < 128

        d_in = {
            # wrapped int16 index layout: slot j -> [j%16, j//16], replicated x8
            "idx": np.ascontiguousarray(
                np.tile(idx.reshape(-1, 16).T, (8, 1)).astype(np.int16)
            ),
            "dstl": np.ascontiguousarray(dstl.reshape(-1, 128).T.astype(F32)),
            "degp": np.ascontiguousarray(degp.reshape(-1, 128).T),
        }
        ins.append(d_in)

    # shared constants
    iota = np.tile(np.arange(128, dtype=F32), (128, 1)).astype(BF16)
    ident = np.eye(128, dtype=F32).astype(BF16)
    wc1 = np.zeros((DF, DF), dtype=F32)
    wc1[:D, :D] = W1
    wc1[D:, D:] = W1
    wc2 = np.zeros((DF, DF), dtype=F32)
    wc2[:D, :D] = W2
    wc2[D:, D:] = W2
    bc1 = np.concatenate([b1, b1]).astype(F32).reshape(DF, 1)
    bc2 = np.concatenate([b2, b2]).astype(F32).reshape(DF, 1)
    wstack = np.zeros((D, DF), dtype=F32)
    wstack[:, :D] = W_dgi.T
    wstack[:, D:] = W_dgi.T
    colmask = np.zeros((DF, 2), dtype=F32)
    colmask[:D, 0] = 1.0
    colmask[D:, 1] = 1.0
    nvalid_last = g.npc - (g.nw - 1) * 128
    lastmask = np.tile((np.arange(128) < nvalid_last).astype(F32), (128, 1))
    mk = (np.arange(g.ldim) < g.npc).astype(F32)
    shared = {
        "x2": X2,
        "iota": iota,
        "ident": ident,
        "wc1": wc1,
        "wc2": wc2,
        "bc1": bc1,
        "bc2": bc2,
        "wstack": wstack,
        "colmask": colmask,
        "lastmask": lastmask,
        "mask": np.ascontiguousarray(mk.reshape(g.nw, 128).T),
        "ones": np.ones((128, 1), dtype=F32),
    }
    for d_in in ins:
        d_in.update(shared)
    struct = (tuple(map(tuple, T)), tuple(map(tuple, calls)))
    return ins, struct


def _build(g, struct):
    T, calls = struct
    T = [list(r) for r in T]
    calls = [list(r) for r in calls]
    tiles_s = [sum(r) for r in T]
    ntiles = sum(tiles_s)
    tbase = [0]
    for s in range(NSEC):
        tbase.append(tbase[-1] + tiles_s[s])

    dt = mybir.dt
    nc = bacc.Bacc(
        "TRN2", target_bir_lowering=False, debug=False, num_devices=C
    )

    def din(name, shape, dty):
        return nc.dram_tensor(name, list(shape), dty, kind="ExternalInput").ap()

    x2 = din("x2", (g.xrows, DF), dt.bfloat16)
    idx_d = din("idx", (128, ntiles * 8), dt.int16)
    dstl_d = din("dstl", (128, ntiles), dt.float32)
    degp_d = din("degp", (128, ntiles), dt.float32)
    iota_d = din("iota", (128, 128), dt.bfloat16)
    ident_d = din("ident", (128, 128), dt.bfloat16)
    wc_d = [din("wc1", (DF, DF), dt.float32), din("wc2", (DF, DF), dt.float32)]
    bc_d = [din("bc1", (DF, 1), dt.float32), din("bc2", (DF, 1), dt.float32)]
    wstack_d = din("wstack", (D, DF), dt.float32)
    colmask_d = din("colmask", (DF, 2), dt.float32)
    lastmask_d = din("lastmask", (128, 128), dt.float32)
    mask_d = din("mask", (128, g.nw), dt.float32)
    ones_d = din("ones", (128, 1), dt.float32)
    loss_out = nc.dram_tensor("loss", [1, 16], dt.float32, kind="ExternalOutput").ap()

    inv_n = 1.0 / float(g.nreal)
    rg = [list(range(C))]

    with tile.TileContext(nc) as tc, ExitStack() as ctx:
        dram = ctx.enter_context(tc.tile_pool(name="dram", bufs=1, space="DRAM"))
        r2shard = []
        for j in range(NSEC):
            r2s_j = dram.tile(
                [g.ql, DF], dt.bfloat16, tag=f"r2shard{j}", name=f"r2shard_{j}"
            )
            r2shard.append(r2s_j)
        r2full = []
        for j in range(NSEC):
            r2f_j = dram.tile(
                [g.sec, DF], dt.bfloat16, tag=f"r2full{j}",
                addr_space="Shared", name=f"r2full_{j}",
            )
            r2full.append(r2f_j)
        cs_in = dram.tile([128, 1], dt.float32, tag="cs_in")
        cs_out = dram.tile([128, 1], dt.float32, tag="cs_out", addr_space="Shared")
        ls_in = dram.tile([1, 16], dt.float32, tag="ls_in")
        ls_out = dram.tile([1, 16], dt.float32, tag="ls_out", addr_space="Shared")

        const = ctx.enter_context(tc.tile_pool(name="const", bufs=1))

        def cload(ap_dram, shape, dty, tag):
            t = const.tile(list(shape), dty, tag=tag)
            nc.sync.dma_start(t[:], ap_dram)
            return t

        iota_sb = cload(iota_d, (128, 128), dt.bfloat16, "iota")
        ident_sb = cload(ident_d, (128, 128), dt.bfloat16, "ident")
        wc_sb = [
            cload(wc_d[0], (DF, DF), dt.float32, "wc1"),
            cload(wc_d[1], (DF, DF), dt.float32, "wc2"),
        ]
        bc_sb = [
            cload(bc_d[0], (DF, 1), dt.float32, "bc1"),
            cload(bc_d[1], (DF, 1), dt.float32, "bc2"),
        ]
        wstack_sb = cload(wstack_d, (D, DF), dt.float32, "wstack")
        colmask_sb = cload(colmask_d, (DF, 2), dt.float32, "colmask")
        lastmask_sb = cload(lastmask_d, (128, 128), dt.float32, "lastmask")
        mask_sb = cload(mask_d, (128, g.nw), dt.float32, "mask")
        ones_sb = cload(ones_d, (128, 1), dt.float32, "ones")

        big = ctx.enter_context(tc.tile_pool(name="big", bufs=1))
        z_sb = big.tile([128, g.ldim], dt.float32, tag="z_sb")
        acc = big.tile([128, g.ldim], dt.float32, tag="acc")
        dl_sb = big.tile([128, ntiles], dt.float32, tag="dl_sb")
        nc.sync.dma_start(dl_sb[:], dstl_d)
        wv_sb = big.tile([128, ntiles], dt.float32, tag="wv_sb")
        nc.sync.dma_start(wv_sb[:], degp_d)
        nc.vector.reciprocal(wv_sb[:], wv_sb[:])
        nc.scalar.sqrt(wv_sb[:], wv_sb[:])

        gpool = ctx.enter_context(tc.tile_pool(name="gpool", bufs=8))
        ipool = ctx.enter_context(tc.tile_pool(name="ipool", bufs=10))
        ppool = ctx.enter_context(tc.tile_pool(name="ppool", bufs=6))
        psw = ctx.enter_context(tc.tile_pool(name="psw", bufs=3, space="PSUM"))
        psm = ctx.enter_context(tc.tile_pool(name="psm", bufs=2, space="PSUM"))
        pst = ctx.enter_context(tc.tile_pool(name="pst", bufs=2, space="PSUM"))
        psl = ctx.enter_context(tc.tile_pool(name="psl", bufs=1, space="PSUM"))
        outp = ctx.enter_context(tc.tile_pool(name="outp", bufs=4))

        # tile prefix per (s, w); call chunks broken at quarter boundaries
        pw = []
        for s in range(NSEC):
            p = [0]
            for w in range(g.nw):
                p.append(p[-1] + T[s][w])
            pw.append(p)
        call_q = [[] for _ in range(NSEC)]    # [s][q] -> list of (t0, nt)
        for s in range(NSEC):
            for q in range(NSEC):
                wlo = q * g.wpq
                whi = min((q + 1) * g.wpq, g.nw)
                t0, t1 = pw[s][wlo], pw[s][whi]
                qc = []
                t = t0
                while t < t1:
                    nt = min(TILES_PER_CALL, t1 - t)
                    qc.append((t, nt))
                    t += nt
                call_q[s].append(qc)
        # first section with tiles, per window
        fsec = [None] * g.nw
        for w in range(g.nw):
            for s in range(NSEC):
                if T[s][w] > 0:
                    fsec[w] = s
                    break

        def post(li, w, emit_ag):
            # out = wc^T @ acc_w; + bias (+relu)
            if fsec[w] is None:
                nc.vector.memset(acc[:, w * 128 : (w + 1) * 128], 0.0)
            po = psm.tile([128, 128], dt.float32, tag="po")
            nc.tensor.matmul(
                po[:],
                lhsT=wc_sb[li][:],
                rhs=acc[:, w * 128 : (w + 1) * 128],
                start=True,
                stop=True,
            )
            sl128 = slice(w * 128, (w + 1) * 128)
            if li == 0:
                rb = outp.tile([128, 128], dt.bfloat16, tag="rb")
                nc.scalar.activation(
                    rb[:],
                    po[:],
                    mybir.ActivationFunctionType.Relu,
                    bias=bc_sb[0][:],
                )
                tp = pst.tile([128, 128], dt.bfloat16, tag="tp")
                nc.tensor.transpose(tp[:], rb[:], ident_sb[:])
                rt = outp.tile([128, 128], dt.bfloat16, tag="rt")
                nc.scalar.activation(
                    rt[:], tp[:], mybir.ActivationFunctionType.Copy
                )
                q = min(w // g.wpq, NSEC - 1)
                lw = w - q * g.wpq
                nc.sync.dma_start(
                    r2shard[q][lw * 128 : (lw + 1) * 128, :], rt[:]
                )
            else:
                nc.vector.tensor_scalar(
                    z_sb[:, sl128],
                    po[:],
                    bc_sb[1][:],
                    None,
                    mybir.AluOpType.add,
                )
                if w == g.nw - 1:
                    nc.vector.tensor_tensor(
                        z_sb[:, sl128],
                        z_sb[:, sl128],
                        lastmask_sb[:],
                        op=mybir.AluOpType.mult,
                    )
            if emit_ag:
                for j in range(NSEC):
                    if w + 1 == min((j + 1) * g.wpq, g.nw):
                        nc.gpsimd.collective_compute(
                            "AllGather",
                            mybir.AluOpType.bypass,
                            replica_groups=rg,
                            ins=[r2shard[j][:].opt()],
                            outs=[r2full[j][:].opt()],
                        )

        def layer(li, src_of, emit_ag):
            gts = {}

            def emit_gathers(s, q):
                for (t0, nt) in call_q[s][q]:
                    it = ipool.tile([128, TILES_PER_CALL * 8], dt.int16, tag="it")
                    nc.sync.dma_start(
                        it[:, : nt * 8],
                        idx_d[:, (tbase[s] + t0) * 8 : (tbase[s] + t0 + nt) * 8],
                    )
                    gt = gpool.tile(
                        [128, TILES_PER_CALL, DF], dt.bfloat16, tag="gt"
                    )
                    nc.gpsimd.dma_gather(
                        gt[:, :nt, :],
                        src_of(s),
                        it[:, : nt * 8],
                        nt * 128,
                        nt * 128,
                        DF,
                        single_packet=False,
                    )
                    gts[(s, t0)] = gt

            def sweep(s, q, do_post):
                wlo = q * g.wpq
                whi = min((q + 1) * g.wpq, g.nw)
                qt0 = pw[s][wlo]
                for w in range(wlo, whi):
                    tw = T[s][w]
                    if tw > 0:
                        ps = psw.tile([128, 128], dt.float32, tag="ps")
                        for t in range(tw):
                            c = pw[s][w] + t
                            cstart = qt0 + ((c - qt0) // TILES_PER_CALL) * TILES_PER_CALL
                            sl = c - cstart
                            col = tbase[s] + c
                            P = ppool.tile([128, 128], dt.bfloat16, tag="P")
                            nc.vector.tensor_scalar(
                                P[:],
                                iota_sb[:],
                                dl_sb[:, col : col + 1],
                                wv_sb[:, col : col + 1],
                                mybir.AluOpType.is_equal,
                                mybir.AluOpType.mult,
                            )
                            nc.tensor.matmul(
                                ps[:],
                                lhsT=gts[(s, cstart)][:, sl, :],
                                rhs=P[:],
                                start=(t == 0),
                                stop=(t == tw - 1),
                            )
                        sl128 = slice(w * 128, (w + 1) * 128)
                        if fsec[w] == s:
                            nc.vector.tensor_copy(acc[:, sl128], ps[:])
                        else:
                            nc.vector.tensor_tensor(
                                acc[:, sl128],
                                acc[:, sl128],
                                ps[:],
                                op=mybir.AluOpType.add,
                            )
                    if do_post:
                        post(li, w, emit_ag)

            # group order: L1 quarter-major (early AGs), L2 section-major
            if li == 0:
                groups = [(q, s) for q in range(NSEC) for s in range(NSEC)]
                order = [(s, q) for (q, s) in groups]
            else:
                order = [(s, q) for s in range(NSEC) for q in range(NSEC)]
            emit_gathers(*order[0])
            for i, (s, q) in enumerate(order):
                if i + 1 < len(order):
                    emit_gathers(*order[i + 1])
                sweep(s, q, do_post=(s == NSEC - 1))

        layer(0, lambda s: x2[s * g.sec : (s + 1) * g.sec, :], emit_ag=True)
        layer(1, lambda s: r2full[s][:], emit_ag=False)

        # ---- DGI readout ----
        fin = ctx.enter_context(tc.tile_pool(name="fin", bufs=1))
        cs = fin.tile([128, 1], dt.float32, tag="cs")
        nc.vector.reduce_sum(cs[:], z_sb[:], axis=mybir.AxisListType.X)
        nc.sync.dma_start(cs_in[:], cs[:])
        nc.gpsimd.collective_compute(
            "AllReduce",
            mybir.AluOpType.add,
            replica_groups=rg,
            ins=[cs_in[:].opt()],
            outs=[cs_out[:].opt()],
        )
        cst = fin.tile([128, 1], dt.float32, tag="cst")
        nc.sync.dma_start(cst[:], cs_out[:])
        summ = fin.tile([128, 1], dt.float32, tag="summ")
        nc.scalar.activation(
            summ[:], cst[:], mybir.ActivationFunctionType.Sigmoid, scale=inv_n
        )
        wsps = psl.tile([DF, 1], dt.float32, tag="pls")
        nc.tensor.matmul(
            wsps[:], lhsT=wstack_sb[:], rhs=summ[0:D, 0:1], start=True, stop=True
        )
        ws2 = fin.tile([DF, 2], dt.float32, tag="ws2")
        nc.vector.tensor_tensor(
            ws2[:],
            colmask_sb[:],
            wsps[:].to_broadcast([DF, 2]),
            op=mybir.AluOpType.mult,
        )
        tp_sb = fin.tile([128, g.nw], dt.float32, tag="tp_sb")
        tn_sb = fin.tile([128, g.nw], dt.float32, tag="tn_sb")
        for dti in range(g.nw):
            sl = slice(dti * 128, (dti + 1) * 128)
            tps = psl.tile([128, 2], dt.float32, tag="pls")
            nc.tensor.matmul(
                tps[:], lhsT=z_sb[:, sl], rhs=ws2[:], start=True, stop=True
            )
            nc.vector.tensor_copy(tp_sb[:, dti : dti + 1], tps[:, 0:1])
            nc.vector.tensor_copy(tn_sb[:, dti : dti + 1], tps[:, 1:2])

        # softplus(sgn*t) = relu(sgn*t) + ln1p(exp(-|t|)); deg-7 poly for ln1p
        LN1P = [
            5.62195900721818e-07, 0.9999574870750696, -0.4992065685478763,
            0.32697310001391783, -0.2228362583278401, 0.13076503250360005,
            -0.05262485136716543, 0.010119082927575069,
        ]

        def softplus_of(t_in, sgn, tagp):
            neg = fin.tile([128, g.nw], dt.float32, tag=f"{tagp}neg")
            nc.vector.tensor_scalar(
                neg[:], t_in[:], -1.0, None, mybir.AluOpType.mult
            )
            ab = fin.tile([128, g.nw], dt.float32, tag=f"{tagp}ab")
            nc.vector.tensor_tensor(ab[:], t_in[:], neg[:], op=mybir.AluOpType.max)
            uu = fin.tile([128, g.nw], dt.float32, tag=f"{tagp}uu")
            nc.scalar.activation(
                uu[:], ab[:], mybir.ActivationFunctionType.Exp, scale=-1.0
            )
            pp_ = fin.tile([128, g.nw], dt.float32, tag=f"{tagp}pp")
            nc.vector.tensor_scalar(
                pp_[:], uu[:], LN1P[7], LN1P[6],
                mybir.AluOpType.mult, mybir.AluOpType.add,
            )
            pm = fin.tile([128, g.nw], dt.float32, tag=f"{tagp}pm")
            for ci in range(5, -1, -1):
                nc.vector.tensor_tensor(
                    pm[:], pp_[:], uu[:], op=mybir.AluOpType.mult
                )
                nc.vector.tensor_scalar(
                    pp_[:], pm[:], LN1P[ci], None, mybir.AluOpType.add
                )
            rl = fin.tile([128, g.nw], dt.float32, tag=f"{tagp}rl")
            nc.vector.tensor_scalar(
                rl[:], (t_in if sgn > 0 else neg)[:], 0.0, None,
                mybir.AluOpType.max,
            )
            res = fin.tile([128, g.nw], dt.float32, tag=f"{tagp}res")
            nc.vector.tensor_tensor(res[:], rl[:], pp_[:], op=mybir.AluOpType.add)
            return res

        spp = softplus_of(tp_sb, -1, "sp")   # softplus(-t_pos)
        spn = softplus_of(tn_sb, +1, "sn")   # softplus(t_neg)
        ssum = fin.tile([128, g.nw], dt.float32, tag="ssum")
        nc.vector.tensor_tensor(ssum[:], spp[:], spn[:], op=mybir.AluOpType.add)
        nc.vector.tensor_tensor(
            ssum[:], ssum[:], mask_sb[:], op=mybir.AluOpType.mult
        )
        srow = fin.tile([128, 1], dt.float32, tag="srow")
        nc.vector.reduce_sum(srow[:], ssum[:], axis=mybir.AxisListType.X)
        tot = psl.tile([1, 1], dt.float32, tag="pls")
        nc.tensor.matmul(
            tot[:], lhsT=srow[:], rhs=ones_sb[:], start=True, stop=True
        )
        lsb = fin.tile([1, 16], dt.float32, tag="lsb")
        nc.vector.memset(lsb[:], 0.0)
        nc.vector.tensor_copy(lsb[0:1, 0:1], tot[:])
        nc.sync.dma_start(ls_in[:], lsb[:])
        nc.gpsimd.collective_compute(
            "AllReduce",
            mybir.AluOpType.add,
            replica_groups=rg,
            ins=[ls_in[:].opt()],
            outs=[ls_out[:].opt()],
        )
        lsf = fin.tile([1, 16], dt.float32, tag="lsf")
        nc.sync.dma_start(lsf[:], ls_out[:])
        lout = fin.tile([1, 16], dt.float32, tag="lout")
        nc.scalar.activation(
            lout[:], lsf[:], mybir.ActivationFunctionType.Copy, scale=inv_n
        )
        nc.sync.dma_start(loss_out, lout[:])

    nc.compile()
    return nc


_prog_cache = {}


def _get_prog(g, struct):
    key = (g.npc, g.nreal, struct)
    if key not in _prog_cache:
        _prog_cache[key] = _build(g, struct)
    return _prog_cache[key]


def run(inputs, npc, nreal, trace=False):
    g = Geo(npc, nreal)
    in_maps, struct = _preprocess(g, **inputs)
    nc = _get_prog(g, struct)
    res = run_bass_kernel_spmd(
        nc, in_maps, core_ids=list(range(C)), trace=trace
    )
    loss = res.results[0]["loss"][0, 0]
    return np.float32(loss), res


def kernel(**inputs):
    out, _ = run(inputs, npc=12500, nreal=100000)
    return out


def _make_sharded_exec(nc, in_maps, reps=1):
    """Reusable jitted shard_map executor mirroring bass2jax's multi-core
    path, with device-resident inputs."""
    import jax
    from jax.experimental.shard_map import shard_map
    from jax.sharding import Mesh, NamedSharding, PartitionSpec

    from concourse import bass2jax, mybir as _mb

    bass2jax.install_neuronx_cc_hook()
    partition_name = (
        nc.partition_id_tensor.name if nc.partition_id_tensor else None
    )
    in_names, out_names, out_avals, zero_shapes = [], [], [], []
    for alloc in nc.m.functions[0].allocations:
        if not isinstance(alloc, _mb.MemoryLocationSet):
            continue
        name = alloc.memorylocations[0].name
        if alloc.kind == "ExternalInput":
            if name != partition_name:
                in_names.append(name)
        elif alloc.kind == "ExternalOutput":
            shape = tuple(alloc.tensor_shape)
            dty = _mb.dt.np(alloc.dtype)
            out_names.append(name)
            out_avals.append(jax.core.ShapedArray(shape, dty))
            zero_shapes.append((shape, dty))
    n_params = len(in_names)
    n_outs = len(out_avals)
    all_names = list(in_names) + list(out_names)
    if partition_name is not None:
        all_names.append(partition_name)
    donate = tuple(range(n_params, n_params + n_outs * reps))

    assert reps == 1  # the neuronx_cc hook allows one bass_exec per module

    def _body(*args):
        operands = list(args)
        if partition_name is not None:
            operands.append(bass2jax.partition_id_tensor())
        outs = bass2jax._bass_exec_p.bind(
            *operands,
            out_avals=tuple(out_avals),
            in_names=tuple(all_names),
            out_names=tuple(out_names),
            lowering_input_output_aliases=(),
            sim_require_finite=True,
            sim_require_nnan=True,
            nc=nc,
        )
        return tuple(outs)

    devices = jax.devices()[:C]
    mesh = Mesh(np.array(devices), ("core",))
    spec = PartitionSpec("core")
    sharded = jax.jit(
        shard_map(
            _body,
            mesh=mesh,
            in_specs=(spec,) * (n_params + n_outs * reps),
            out_specs=(spec,) * n_outs,
            check_rep=False,
        ),
        donate_argnums=donate,
        keep_unused=True,
    )
    shard = NamedSharding(mesh, spec)
    concat_in = [
        jax.device_put(
            np.concatenate([np.asarray(m[nm]) for m in in_maps], axis=0), shard
        )
        for nm in in_names
    ]

    def launch():
        zeros = [
            jax.device_put(np.zeros((C * s[0], *s[1:]), d), shard)
            for (s, d) in zero_shapes
        ]
        return sharded(*concat_in, *zeros)

    def fetch(outs):
        jax.block_until_ready(outs)
        return {
            nm: np.asarray(outs[i]).reshape(C, *out_avals[i].shape)[0]
            for i, nm in enumerate(out_names)
        }

    def run_once():
        return fetch(launch())

    run_once.launch = launch
    run_once.fetch = fetch
    return run_once


def bench(inputs, npc=12500, nreal=100000, iters=6):
    import time

    g = Geo(npc, nreal)
    t0 = time.time()
    in_maps, struct = _preprocess(g, **inputs)
    t1 = time.time()
    nc = _get_prog(g, struct)
    t2 = time.time()
    run_1 = _make_sharded_exec(nc, in_maps)
    out = run_1()  # warmup: compiles + loads NEFF
    t3 = time.time()
    t1s = []
    for _ in range(iters):
        ta = time.time()
        out = run_1()
        t1s.append(time.time() - ta)
    K = 48
    ta = time.time()
    pend = [run_1.launch() for _ in range(K)]
    import jax as _jax
    _jax.block_until_ready(pend)
    tK = time.time() - ta
    per = (tK - min(t1s)) / (K - 1)
    print(
        f"preprocess {t1-t0:.1f}s  build {t2-t1:.1f}s  warmup {t3-t2:.1f}s\n"
        f"  1-shot ms: {[round(t*1e3,2) for t in t1s]}\n"
        f"  {K} pipelined: total {tK*1e3:.1f} ms -> marginal {per*1e3:.3f} ms"
    )
    return np.float32(out["loss"][0, 0]), per
